# revision 13
# baseline (speedup 1.0000x reference)
"""Trainium2 Bass kernel for a pre-norm transformer encoder layer with GEGLU FFN.

Sharding: token-data-parallel over 8 cores. Core c handles batch c//4 and the
512-token slice (c%4) of that batch. K/V for the full 2048-token sequence are
exchanged with one AllGather per 4-core group. All activations are kept
feature-major [feature(partition), token(free)] so the matmul chain composes
with zero on-chip transposes; softmax runs on transposed scores with the
denominator computed by a ones-column in the AV matmul.
"""

import numpy as np

import concourse.bass as bass
import concourse.mybir as mybir
import concourse.tile as tile
from concourse import bacc
from concourse.bass_utils import run_bass_kernel_spmd

P = 128
D = 64  # head dim (fixed)
F32 = mybir.dt.float32
F32R = mybir.dt.float32r
BF16 = mybir.dt.bfloat16
AF = mybir.ActivationFunctionType
ALU = mybir.AluOpType

# full-size problem config
FULL = dict(E=1024, T_OWN=512, FF=4096, R=4)
EPS = 1e-5
N_CORES = 8
GROUPS = [[0, 1, 2, 3], [4, 5, 6, 7]]


def build(nc, E, T_OWN, FF, R):
    H = E // D            # heads
    n_et = E // P         # e-tiles == head-pairs == out-tiles
    n_ff = FF // P        # ff tiles per half (a / gate)
    T = R * T_OWN         # full sequence
    n_tt = T // P         # t2 tiles
    n_to = T_OWN // P     # own-token tiles
    T1 = T_OWN            # free dim of most matmuls (<= 512)
    assert T1 <= 512
    OC_W = min(512, E)
    n_oc = E // OC_W

    # ---- DRAM I/O ----
    # srcT holds the full batch sequence feature-major, with the core's own
    # 512-token chunk rotated to column-block 0 (so one SPMD program serves
    # all cores; softmax ordering over t2 is permutation-invariant).
    OWN_R = 0
    srcT = nc.dram_tensor("srcT", [E, T], F32R, kind="ExternalInput")
    wq = nc.dram_tensor("wq", [n_et, P, n_et, P], F32R, kind="ExternalInput")
    wk = nc.dram_tensor("wk", [n_et, P, n_et, P], F32R, kind="ExternalInput")
    wv = nc.dram_tensor("wv", [n_oc, n_et, P, OC_W], F32R, kind="ExternalInput")
    wo = nc.dram_tensor("wo", [n_et, P, n_et, P], F32R, kind="ExternalInput")
    w1 = nc.dram_tensor("w1", [2 * n_ff, P, n_et, P], F32R, kind="ExternalInput")
    w2 = nc.dram_tensor("w2", [n_et, P, n_ff, P], F32R, kind="ExternalInput")
    b1d = nc.dram_tensor("b1d", [2 * FF, 1], F32, kind="ExternalInput")
    b2d = nc.dram_tensor("b2d", [E, 1], F32, kind="ExternalInput")
    ln1w = nc.dram_tensor("ln1w", [E, 1], F32, kind="ExternalInput")
    ln1b = nc.dram_tensor("ln1b", [E, 1], F32, kind="ExternalInput")
    ln2w = nc.dram_tensor("ln2w", [E, 1], F32, kind="ExternalInput")
    ln2b = nc.dram_tensor("ln2b", [E, 1], F32, kind="ExternalInput")
    outT = nc.dram_tensor("outT", [E, T_OWN], F32, kind="ExternalOutput")

    def mm(ps, lhsT, rhs, start, stop):
        nc.tensor.matmul(ps, lhsT, rhs, start=start, stop=stop)

    with nc.allow_low_precision(reason="f32r/bf16 tiles feeding PE; fp32 PSUM accumulation"), \
            tile.TileContext(nc) as tc, tc.tile_pool(name="consts", bufs=1) as constp:
        def single(shape, name, dt=F32):
            return constp.tile(shape, dt, name=name, tag=name)

        # ---- constants / small params ----
        ones_col = single([P, 1], "ones_col", F32R)
        nc.vector.memset(ones_col[:].bitcast(F32), 1.0)
        ones_row = single([1, P], "ones_row", F32R)
        nc.vector.memset(ones_row[:].bitcast(F32), 1.0)
        eps_t = single([1, 1], "eps_t")
        nc.vector.memset(eps_t[:], EPS)

        lnc = single([P, 4 * n_et], "lnc")
        nc.sync.dma_start(lnc[:, 0 * n_et:1 * n_et], ln1w.rearrange("(c p) o -> p (c o)", p=P))
        nc.sync.dma_start(lnc[:, 1 * n_et:2 * n_et], ln1b.rearrange("(c p) o -> p (c o)", p=P))
        nc.sync.dma_start(lnc[:, 2 * n_et:3 * n_et], ln2w.rearrange("(c p) o -> p (c o)", p=P))
        nc.sync.dma_start(lnc[:, 3 * n_et:4 * n_et], ln2b.rearrange("(c p) o -> p (c o)", p=P))
        b1c = single([P, 2 * n_ff], "b1c")
        nc.sync.dma_start(b1c[:], b1d.rearrange("(c p) o -> p (c o)", p=P))
        b2c = single([P, n_et], "b2c")
        nc.sync.dma_start(b2c[:], b2d.rearrange("(c p) o -> p (c o)", p=P))

        from contextlib import ExitStack
        with ExitStack() as es:
            pool = lambda **kw: es.enter_context(tc.tile_pool(**kw))
            srcp = pool(name="srcp", bufs=10)
            hTp = pool(name="hT", bufs=12)              # h1 chunks / attnT / h2
            qp = pool(name="qp", bufs=n_et)             # qT bf16
            bigp = pool(name="big", bufs=n_ff)          # fT
            xp = pool(name="xp", bufs=n_et)
            wblkp = pool(name="wblk", bufs=3)
            wvp = pool(name="wvp", bufs=3)
            w2p = pool(name="w2p", bufs=2)
            kslabp = pool(name="kslab", bufs=2)
            vsbp = pool(name="vsb", bufs=2)
            probsp = pool(name="probs", bufs=3)
            evp = pool(name="ev", bufs=3)
            rbp = pool(name="rb", bufs=2)
            statsp = pool(name="stats", bufs=4)
            psS = pool(name="psS", bufs=3, space="PSUM")
            psA = pool(name="psA", bufs=2, space="PSUM")
            psM = pool(name="psM", bufs=3, space="PSUM")
            dram = pool(name="dram", bufs=1, space="DRAM")

            # local DRAM scratch for full-sequence K (feature-major) and V (head-major)
            kT_dram = dram.tile([E, T], BF16, name="kT_dram")
            v_dram = dram.tile([H, T, D], BF16, name="v_dram")

            def layer_norm(x_tiles, wcol, bcol, label, out_pool, width):
                """x_tiles: n_et SBUF [P, width] feature-major chunk."""
                m_ps = psM.tile([1, width], F32, name=f"mps_{label}", tag="psM")
                s_ps = psM.tile([1, width], F32, name=f"sps_{label}", tag="psM")
                for kt in range(n_et):
                    mm(m_ps[:], ones_col[:], x_tiles[kt][:], kt == 0, kt == n_et - 1)
                for kt in range(n_et):
                    sq = evp.tile([P, width], F32R, name=f"sq_{label}{kt}", tag="ev")
                    nc.scalar.square(sq[:], x_tiles[kt][:])
                    mm(s_ps[:], ones_col[:], sq[:], kt == 0, kt == n_et - 1)
                m_sb = statsp.tile([1, width], F32R, name=f"m_{label}", tag="st")
                nc.vector.tensor_scalar_mul(m_sb[:], m_ps[:], 1.0 / E)
                ms_sb = statsp.tile([1, width], F32, name=f"ms_{label}", tag="st")
                nc.vector.tensor_scalar_mul(ms_sb[:], s_ps[:], 1.0 / E)
                mm_sb = statsp.tile([1, width], F32, name=f"mm_{label}", tag="st")
                nc.vector.tensor_mul(mm_sb[:], m_sb[:], m_sb[:])
                var = statsp.tile([1, width], F32, name=f"var_{label}", tag="st")
                nc.vector.tensor_sub(var[:], ms_sb[:], mm_sb[:])
                sd = statsp.tile([1, width], F32, name=f"sd_{label}", tag="st")
                nc.scalar.activation(sd[:], var[:], AF.Sqrt, bias=eps_t[0:1, 0:1])
                rinv = statsp.tile([1, width], F32R, name=f"ri_{label}", tag="st")
                nc.vector.reciprocal(rinv[:], sd[:])
                # broadcast mean and rstd across partitions via K=1 matmuls
                mb_ps = psM.tile([P, width], F32, name=f"mb_{label}", tag="psM")
                mm(mb_ps[:], ones_row[0:1, :], m_sb[:], True, True)
                rs_ps = psM.tile([P, width], F32, name=f"rb_{label}", tag="psM")
                mm(rs_ps[:], ones_row[0:1, :], rinv[:], True, True)
                rstd_b = evp.tile([P, width], F32, name=f"rstdb_{label}", tag="ev")
                nc.vector.tensor_copy(rstd_b[:], rs_ps[:])
                h_tiles = []
                for kt in range(n_et):
                    t = evp.tile([P, width], F32, name=f"xc_{label}{kt}", tag="ev")
                    nc.vector.tensor_sub(t[:], x_tiles[kt][:], mb_ps[:])
                    t2 = evp.tile([P, width], F32, name=f"xs_{label}{kt}", tag="ev")
                    nc.vector.scalar_tensor_tensor(
                        t2[:], t[:], wcol[:, kt:kt + 1], rstd_b[:], ALU.mult, ALU.mult)
                    h = out_pool.tile([P, width], F32R, name=f"h_{label}{kt}", tag="hT")
                    nc.vector.tensor_scalar_add(h[:], t2[:], bcol[:, kt:kt + 1])
                    h_tiles.append(h)
                return h_tiles

            # ---- LN1 + K/V over full sequence, chunked; Q for own tokens ----
            # own chunk index is baked into the data: srcT column-block OWN_R
            q_sb = [None] * n_et
            for r in range(R):
                xs = []
                for kt in range(n_et):
                    sx = srcp.tile([P, T1], F32R, name=f"src{r}_{kt}", tag="src")
                    nc.sync.dma_start(sx[:], srcT[kt * P:(kt + 1) * P,
                                                  r * T1:(r + 1) * T1])
                    xs.append(sx)
                h1r = layer_norm(xs, lnc[:, 0:n_et], lnc[:, n_et:2 * n_et],
                                 f"l1c{r}", hTp, T1)
                # K chunk: kT[:, r-block]
                for ot in range(n_et):
                    wk_sb = wblkp.tile([P, n_et, P], F32R, name=f"wk{r}_{ot}", tag="wblk")
                    nc.sync.dma_start(wk_sb[:], wk[ot])
                    ps = psM.tile([P, T1], F32, name=f"psk{r}_{ot}", tag="psM")
                    for kt in range(n_et):
                        mm(ps[:], wk_sb[:, kt, :], h1r[kt][:], kt == 0, kt == n_et - 1)
                    kev = evp.tile([P, T1], BF16, name=f"kev{r}_{ot}", tag="evb")
                    nc.vector.tensor_copy(kev[:], ps[:])
                    nc.sync.dma_start(
                        kT_dram[ot * P:(ot + 1) * P, r * T1:(r + 1) * T1], kev[:])
                # V chunk: v[heads, r-block, :]
                for oc in range(n_oc):
                    for to in range(n_to):
                        ps = psM.tile([P, OC_W], F32, name=f"psv{r}_{oc}_{to}", tag="psM")
                        for kt in range(n_et):
                            wv_sb = wvp.tile([P, OC_W], F32R,
                                             name=f"wv{r}_{oc}_{to}_{kt}", tag="wv")
                            nc.sync.dma_start(wv_sb[:], wv[oc, kt])
                            mm(ps[:], h1r[kt][:, to * P:(to + 1) * P], wv_sb[:],
                               kt == 0, kt == n_et - 1)
                        vev = evp.tile([P, OC_W], BF16, name=f"vev{r}_{oc}_{to}", tag="evb")
                        nc.vector.tensor_copy(vev[:], ps[:])
                        for hh in range(OC_W // D):
                            h_idx = oc * (OC_W // D) + hh
                            nc.sync.dma_start(
                                v_dram[h_idx, r * T1 + to * P: r * T1 + (to + 1) * P, :],
                                vev[:, hh * D:(hh + 1) * D])
                # Q only for the OWN chunk (srcT own block marked by prep: always block OWN_R)
                if r == OWN_R:
                    for ot in range(n_et):
                        wq_sb = wblkp.tile([P, n_et, P], F32R, name=f"wq{ot}", tag="wblk")
                        nc.sync.dma_start(wq_sb[:], wq[ot])
                        ps = psM.tile([P, T1], F32, name=f"psq{ot}", tag="psM")
                        for kt in range(n_et):
                            mm(ps[:], wq_sb[:, kt, :], h1r[kt][:], kt == 0, kt == n_et - 1)
                        q = qp.tile([P, T1], BF16, name=f"q{ot}", tag="q")
                        nc.vector.tensor_copy(q[:], ps[:])
                        q_sb[ot] = q

            # ---- attention, one head-pair (=128 feature rows) at a time ----
            attn_sb = []
            for hp in range(n_et):
                kslab = kslabp.tile([P, T], BF16, name=f"ks{hp}", tag="ks")
                nc.sync.dma_start(kslab[:], kT_dram[hp * P:(hp + 1) * P, :])
                vsb = []
                for hl in range(2):
                    h_idx = hp * 2 + hl
                    v = vsbp.tile([P, n_tt, D + 1], BF16, name=f"v{hp}_{hl}", tag="vs")
                    nc.sync.dma_start(
                        v[:, :, 0:D],
                        v_dram[h_idx].rearrange("(tt p) d -> p tt d", p=P))
                    nc.vector.memset(v[:, :, D:D + 1], 1.0)
                    vsb.append(v)
                att_ps = [psA.tile([D + 1, T1], F32, name=f"pa{hp}_{hl}", tag="psA")
                          for hl in range(2)]
                for tt in range(n_tt):
                    for hl in range(2):
                        sc = psS.tile([P, T1], F32, name=f"sc{hp}_{tt}_{hl}", tag="psS")
                        mm(sc[:],
                           kslab[hl * D:(hl + 1) * D, tt * P:(tt + 1) * P],
                           q_sb[hp][hl * D:(hl + 1) * D, :], True, True)
                        pr = probsp.tile([P, T1], BF16, name=f"pr{hp}_{tt}_{hl}", tag="pr")
                        nc.scalar.activation(pr[:], sc[:], AF.Exp, scale=0.125)
                        mm(att_ps[hl][:], vsb[hl][:, tt, :], pr[:],
                           tt == 0, tt == n_tt - 1)
                a = hTp.tile([P, T1], F32R, name=f"attn{hp}", tag="hT")
                for hl in range(2):
                    rec = statsp.tile([1, T1], F32R, name=f"rec{hp}_{hl}", tag="st")
                    nc.vector.reciprocal(rec[:], att_ps[hl][D:D + 1, :])
                    rbps = psS.tile([P, T1], F32, name=f"rbp{hp}_{hl}", tag="psS")
                    mm(rbps[0:D, :], ones_row[0:1, 0:D], rec[:], True, True)
                    rb_sb = rbp.tile([D, T1], F32, name=f"rbs{hp}_{hl}", tag="rb")
                    nc.vector.tensor_copy(rb_sb[:], rbps[0:D, :])
                    nc.vector.tensor_mul(a[hl * D:(hl + 1) * D, :],
                                         att_ps[hl][0:D, :], rb_sb[:])
                attn_sb.append(a)

            # ---- Wo + residual -> xT ----
            x_sb = []
            for ot in range(n_et):
                wo_sb = wblkp.tile([P, n_et, P], F32R, name=f"wo{ot}", tag="wblk")
                nc.sync.dma_start(wo_sb[:], wo[ot])
                ps = psM.tile([P, T1], F32, name=f"pso{ot}", tag="psM")
                for kt in range(n_et):
                    mm(ps[:], wo_sb[:, kt, :], attn_sb[kt][:], kt == 0, kt == n_et - 1)
                so = evp.tile([P, T1], F32R, name=f"so{ot}", tag="ev")
                nc.sync.dma_start(so[:], srcT[ot * P:(ot + 1) * P,
                                              OWN_R * T1:(OWN_R + 1) * T1])
                x = xp.tile([P, T1], F32R, name=f"x{ot}", tag="x")
                nc.vector.tensor_add(x[:], ps[:], so[:])
                x_sb.append(x)

            # ---- LN2 ----
            h2 = layer_norm(x_sb, lnc[:, 2 * n_et:3 * n_et],
                            lnc[:, 3 * n_et:4 * n_et], "l2", hTp, T1)

            # ---- FFN: u = h2 @ W1.T + b1 (transposed), GEGLU ----
            f_sb = []
            for pt in range(n_ff):
                w1a = wblkp.tile([P, n_et, P], F32R, name=f"w1a{pt}", tag="wblk")
                nc.sync.dma_start(w1a[:], w1[pt])
                w1g = wblkp.tile([P, n_et, P], F32R, name=f"w1g{pt}", tag="wblk")
                nc.sync.dma_start(w1g[:], w1[n_ff + pt])
                psa = psM.tile([P, T1], F32, name=f"psa{pt}", tag="psM")
                for kt in range(n_et):
                    mm(psa[:], w1a[:, kt, :], h2[kt][:], kt == 0, kt == n_et - 1)
                psg = psM.tile([P, T1], F32, name=f"psg{pt}", tag="psM")
                for kt in range(n_et):
                    mm(psg[:], w1g[:, kt, :], h2[kt][:], kt == 0, kt == n_et - 1)
                gel = evp.tile([P, T1], F32, name=f"gel{pt}", tag="ev")
                nc.scalar.activation(gel[:], psg[:], AF.Gelu,
                                     bias=b1c[:, n_ff + pt:n_ff + pt + 1])
                f = bigp.tile([P, T1], F32R, name=f"f{pt}", tag="big")
                nc.vector.scalar_tensor_tensor(
                    f[:], psa[:], b1c[:, pt:pt + 1], gel[:], ALU.add, ALU.mult)
                f_sb.append(f)

            # ---- W2 + b2 + residual -> outT ----
            n_ffh = max(1, n_ff // 4)
            for ot in range(n_et):
                ps = psM.tile([P, T1], F32, name=f"psy{ot}", tag="psM")
                w2h = []
                for half in range(n_ff // n_ffh):
                    w = w2p.tile([P, n_ffh, P], F32R, name=f"w2_{ot}_{half}", tag="w2")
                    nc.sync.dma_start(
                        w[:], w2[ot, :, half * n_ffh:(half + 1) * n_ffh, :])
                    w2h.append(w)
                for c in range(n_ff):
                    mm(ps[:], w2h[c // n_ffh][:, c % n_ffh, :], f_sb[c][:],
                       c == 0, c == n_ff - 1)
                y = evp.tile([P, T1], F32, name=f"y{ot}", tag="ev")
                nc.vector.scalar_tensor_tensor(
                    y[:], ps[:], b2c[:, ot:ot + 1], x_sb[ot][:], ALU.add, ALU.add)
                nc.sync.dma_start(outT[ot * P:(ot + 1) * P, :], y[:])

    return nc


def prep_inputs(src, Wq, Wk, Wv, Wo, W1, b1, W2, b2,
                ln1_w, ln1_b, ln2_w, ln2_b, E, T_OWN, FF, R):
    """Host-side: transpose/retile weights, shard src. Returns per-core in_maps."""
    n_et = E // P
    n_ff = FF // P
    OC_W = min(512, E)
    n_oc = E // OC_W
    c = np.ascontiguousarray
    shared = {
        "wq": c(Wq.reshape(n_et, P, n_et, P).transpose(0, 3, 2, 1)),
        "wk": c(Wk.reshape(n_et, P, n_et, P).transpose(0, 3, 2, 1)),
        "wv": c(Wv.reshape(n_oc, OC_W, n_et, P).transpose(0, 2, 3, 1)),
        "wo": c(Wo.reshape(n_et, P, n_et, P).transpose(0, 3, 2, 1)),
        "w1": c(W1.reshape(2 * n_ff, P, n_et, P).transpose(0, 3, 2, 1)),
        "w2": c(W2.reshape(n_et, P, n_ff, P).transpose(0, 3, 2, 1)),
        "b1d": c(b1.reshape(2 * FF, 1)),
        "b2d": c(b2.reshape(E, 1)),
        "ln1w": c(ln1_w.reshape(E, 1)),
        "ln1b": c(ln1_b.reshape(E, 1)),
        "ln2w": c(ln2_w.reshape(E, 1)),
        "ln2b": c(ln2_b.reshape(E, 1)),
    }
    in_maps = []
    for core in range(N_CORES):
        b, r = core // R, core % R
        order = [r] + [x for x in range(R) if x != r]
        blocks = [src[b, x * T_OWN:(x + 1) * T_OWN, :].T for x in order]
        m = dict(shared)
        m["srcT"] = c(np.concatenate(blocks, axis=1))
        in_maps.append(m)
    return in_maps


_CACHE = {}


def _compiled(cfg_key):
    if cfg_key not in _CACHE:
        E, T_OWN, FF, R = cfg_key
        nc = bacc.Bacc("TRN2", target_bir_lowering=False, debug=False,
                       num_devices=N_CORES)
        build(nc, E, T_OWN, FF, R)
        nc.compile()
        _CACHE[cfg_key] = nc
    return _CACHE[cfg_key]


def run(inputs, cfg, trace=False, tmpdir=None, trace_cores=None):
    E, T_OWN, R = cfg["E"], cfg["T_OWN"], cfg["R"]
    nc = _compiled((E, T_OWN, cfg["FF"], R))
    in_maps = prep_inputs(
        np.asarray(inputs["src"], np.float32),
        np.asarray(inputs["Wq"], np.float32), np.asarray(inputs["Wk"], np.float32),
        np.asarray(inputs["Wv"], np.float32), np.asarray(inputs["Wo"], np.float32),
        np.asarray(inputs["W1"], np.float32), np.asarray(inputs["b1"], np.float32),
        np.asarray(inputs["W2"], np.float32), np.asarray(inputs["b2"], np.float32),
        np.asarray(inputs["ln1_w"], np.float32), np.asarray(inputs["ln1_b"], np.float32),
        np.asarray(inputs["ln2_w"], np.float32), np.asarray(inputs["ln2_b"], np.float32),
        E, T_OWN, cfg["FF"], R)
    res = run_bass_kernel_spmd(nc, in_maps, core_ids=list(range(N_CORES)),
                               trace=trace, tmpdir=tmpdir, trace_cores=trace_cores)
    B, T = 8 // R, R * T_OWN
    out = np.empty((B, T, E), np.float32)
    for core in range(N_CORES):
        b, r = core // R, core % R
        out[b, r * T_OWN:(r + 1) * T_OWN, :] = res.results[core]["outT"].T
    return out, res


def kernel(**inputs) -> np.ndarray:
    out, _ = run(inputs, FULL)
    return out
